# revision 7
# baseline (speedup 1.0000x reference)
"""BitNet attention (D_MODEL=2048, 16 heads, B=2, T=2048) on 8 TRN2 cores, v2.

Tensor-parallel over heads: each core owns 2 heads (256 dims) of q/k/v
(column-parallel) and 256 output columns of out_proj on an AllGather-ed
int8 activation.

v2 restructure vs v1:
- qx transposes via big XBAR DMAs per (i, t4) [512,128]->[128,512] instead of
  256 tiny per-(i,j) DMAs.
- bf16 QT/KT/V/PVT/ex (halves SBUF, full PE rate) -> both batches' pipelines
  overlap; the AllReduce(max)/AllGather of batch b hide under batch 1-b.
- AllGather payload is int8 (4x smaller), decoded by gpsimd cast-DMAs into
  the qx tile buffers (tag-shared, WAR-rotated).
- softmax denominators: ex-pair sums on DVE/Pool + 8 short matmuls per
  (head, t1-block) instead of 16 full ones-matmuls.
- V computed as VT (weight-stationary streams) + PE transpose.
- Cross-batch buffer reuse via shared tile-pool tags (bufs=1 rotation), so
  pool lifetimes stay stack-nested as the Tile allocator requires.
"""

import os
import numpy as np

import concourse.bass as bass
import concourse.mybir as mybir
import concourse.tile as tile
from concourse.bass_utils import run_bass_kernel_spmd
from concourse.vector_clock import ScopedClock

DT = mybir.dt
ALU = mybir.AluOpType
ACTF = mybir.ActivationFunctionType

N_CORES = 8
P = 128
FD = 2048          # d_model
B, T = 2, 2048
BT = B * T
OC = FD // N_CORES  # 256 per-core head dims / out_proj cols
NF = FD // P        # 16 feature tiles
NT = T // P         # 16 token tiles per batch
TB = 512            # token block
NTB = T // TB       # 4
MAGIC = 12582912.0  # 1.5 * 2**23
RG = [list(range(N_CORES))]

# ---------------------------------------------------------------------------
# Workaround: the bundled walrus rejects >1 sem-wait on a Drain (CTRL_NO_STRUCT)
# instruction. Split the TileContext tail drain into single-wait drains.
_orig_drain_and_barrier = tile.TileContext._drain_and_barrier


def _patched_drain_and_barrier(self, tick_clock, wait_clock):
    nc = self.nc
    drain_inst = nc.sync.drain()
    wait_clock.add_sem_waits(
        drain_inst.ins, ScopedClock({None: tick_clock.global_clock})
    )
    si = drain_inst.ins.sync_info
    waits = list(si.on_wait or []) if si is not None else []
    if len(waits) > 1:
        si.on_wait = waits[:1]
        for w in waits[1:]:
            extra = nc.sync.drain()
            extra.ins.sync_info = mybir.SyncInfo(on_wait=[w], on_update=[])

    nc.all_engine_barrier()
    assert self.sems is not None
    popped = nc._tile_sem_poison_stack.pop()
    assert popped is self._sem_poison
    nc.clear_and_free_semaphores(list(self.sems.allocated().values()))
    nc.all_engine_barrier()


def _install_patch():
    tile.TileContext._drain_and_barrier = _patched_drain_and_barrier


def _split_excess_waits(nc, max_waits):
    n_new = 0
    for fn in nc.m.functions:
        for blk in fn.blocks:
            il = blk.instructions
            out = []
            for inst in il:
                si = getattr(inst, "sync_info", None)
                waits = list(si.on_wait) if (si is not None and si.on_wait) else []
                if len(waits) > max_waits:
                    extra = waits[:-max_waits] if max_waits else waits
                    keep = waits[-max_waits:] if max_waits else []
                    step = max(1, max_waits)
                    for k in range(0, len(extra), step):
                        n_new += 1
                        nop = mybir.InstNoOp(
                            name=f"WSP{n_new}",
                            sync_info=mybir.SyncInfo(
                                on_wait=extra[k:k + step], on_update=[]),
                            bass_nofuse=True,
                            engine=inst.engine,
                        )
                        nc.register_instruction(nop, overwrite=True)
                        out.append(nop)
                    si.on_wait = keep
                out.append(inst)
            il[:] = out
    return n_new


# ---------------------------------------------------------------------------


def build_kernel():
    _install_patch()
    nc = bass.Bass("TRN2", target_bir_lowering=False, debug=False,
                   num_devices=N_CORES)
    x_in = nc.dram_tensor("x", [BT, FD], DT.float32, kind="ExternalInput")
    wT = {
        w: nc.dram_tensor(f"w{w}T", [FD, OC], DT.float32, kind="ExternalInput")
        for w in "qkvo"
    }
    y_out = nc.dram_tensor("y", [B, T, OC], DT.float32, kind="ExternalOutput")

    with tile.TileContext(nc) as tc:
        for _rep in range(int(os.environ.get("KREPEAT", "1"))):
            _body(nc, tc, x_in, wT, y_out)
    _split_excess_waits(nc, int(os.environ.get("BASS_MAX_WAITS", "1")))
    return nc


class St:
    """Per-batch emission state (tile views into shared-tag buffers)."""
    def __init__(self):
        self.t = {}


def _body(nc, tc, x_in, wT, y_out):
    fp32 = DT.float32
    bf16 = DT.bfloat16
    X = mybir.AxisListType.X

    from contextlib import ExitStack
    stack = ExitStack()
    const = stack.enter_context(tc.tile_pool(name="const", bufs=1))
    dram = stack.enter_context(tc.tile_pool(name="dram", bufs=1, space="DRAM"))
    big = stack.enter_context(tc.tile_pool(name="big", bufs=1))

    ones_row = const.tile([1, P], fp32, tag="ones_row", name="ones_row")
    nc.gpsimd.memset(ones_row[:], 1.0)
    ones_col = const.tile([P, 1], fp32, tag="ones_col", name="ones_col")
    nc.gpsimd.memset(ones_col[:], 1.0)
    ones_col_r = const.tile([P, 1], DT.float32r, tag="ones_col_r",
                            name="ones_col_r")
    nc.vector.tensor_copy(ones_col_r[:], ones_col[:])
    ident = const.tile([P, P], fp32, tag="ident", name="ident")
    from concourse.masks import make_identity
    make_identity(nc, ident[:])
    identb = const.tile([P, P], bf16, tag="identb", name="identb")
    nc.vector.tensor_copy(identb[:], ident[:])
    identr = const.tile([P, P], DT.float32r, tag="identr", name="identr")
    nc.vector.tensor_copy(identr[:], ident[:])
    magicv = const.tile([P, 1], fp32, tag="magicv", name="magicv")
    nc.gpsimd.memset(magicv[:], MAGIC)

    # quantized weights (persistent): 4 x 16 tiles [128, 256] bf16
    wqbf = {
        w: [const.tile([P, OC], bf16, tag=f"w{w}bf{i}", name=f"w{w}bf{i}")
            for i in range(NF)]
        for w in "qkvo"
    }
    swb = const.tile([P, 8], fp32, tag="swb", name="swb")  # 0-3 s_w, 4-7 1/s_w
    cvec = const.tile([P, 1], fp32, tag="cvec", name="cvec")

    # ---------------- P0: weight quantization ----------------
    with (
        tc.tile_pool(name="wstage", bufs=3) as wstage,
        tc.tile_pool(name="p0", bufs=1) as p0,
        tc.tile_pool(name="p0ps", bufs=2, space="PSUM") as p0ps,
    ):
        asum = p0.tile([P, 4 * NF], fp32, tag="asum", name="asum")
        tots = p0.tile([P, 4], fp32, tag="tots", name="tots")
        for wi, w in enumerate("qkvo"):
            for i in range(NF):
                st = wstage.tile([P, OC], fp32, tag="wst", name="wst")
                nc.sync.dma_start(out=st[:], in_=wT[w][i * P:(i + 1) * P, :])
                nc.vector.tensor_reduce(
                    asum[:, wi * NF + i: wi * NF + i + 1], st[:],
                    X, ALU.add, apply_absolute_value=True,
                )
            nc.vector.tensor_reduce(
                tots[:, wi:wi + 1], asum[:, wi * NF:(wi + 1) * NF], X, ALU.add,
            )
        ps14 = p0ps.tile([1, 4], fp32, tag="ps14", name="ps14")
        nc.tensor.matmul(ps14[:], lhsT=ones_col[:], rhs=tots[:],
                         start=True, stop=True)
        sums4 = p0.tile([1, 4], fp32, tag="sums4", name="sums4")
        nc.vector.tensor_copy(sums4[:], ps14[:])
        cc_in = dram.tile([1, 4], fp32, tag="cc_in", name="cc_in")
        cc_out = dram.tile([1, 4], fp32, tag="cc_out", name="cc_out")
        nc.sync.dma_start(out=cc_in[:], in_=sums4[:])
        nc.gpsimd.collective_compute(
            "AllReduce", ALU.add, replica_groups=RG,
            ins=[cc_in.opt()], outs=[cc_out.opt()],
        )
        row8 = p0.tile([1, 8], fp32, tag="row8", name="row8")
        gs = p0.tile([1, 4], fp32, tag="gs", name="gs")
        nc.sync.dma_start(out=gs[:], in_=cc_out[:])
        nc.vector.tensor_scalar(row8[:, 4:8], gs[:], 1.0 / (FD * FD), 1e-5,
                                ALU.mult, ALU.max)
        nc.vector.reciprocal(row8[:, 0:4], row8[:, 4:8])
        psb = p0ps.tile([P, 8], fp32, tag="psb", name="psb")
        nc.tensor.matmul(psb[:], lhsT=ones_row[:], rhs=row8[:],
                         start=True, stop=True)
        nc.vector.tensor_copy(swb[:], psb[:])
        cv1 = p0.tile([P, 1], fp32, tag="cv1", name="cv1")
        nc.vector.tensor_tensor(cv1[:], swb[:, 4:5], swb[:, 5:6], ALU.mult)
        nc.vector.tensor_scalar_mul(cvec[:], cv1[:], float(P) ** -0.5)

        # quantize: round(w * s_w) clipped to [-1, 1], bf16
        for wi, w in enumerate("qkvo"):
            for i in range(NF):
                st = wstage.tile([P, OC], fp32, tag="wst", name="wst")
                nc.sync.dma_start(out=st[:], in_=wT[w][i * P:(i + 1) * P, :])
                t1 = wstage.tile([P, OC], fp32, tag="wt1", name="wt1")
                nc.scalar.activation(t1[:], st[:], ACTF.Identity,
                                     scale=swb[:, wi:wi + 1], bias=magicv[:])
                t2 = wstage.tile([P, OC], fp32, tag="wt2", name="wt2")
                nc.vector.tensor_scalar(t2[:], t1[:], -MAGIC, 1.0,
                                        ALU.add, ALU.min)
                nc.vector.tensor_scalar_max(wqbf[w][i][:], t2[:], -1.0)

    shared = dict(nc=nc, tc=tc, x_in=x_in, y_out=y_out, dram=dram, big=big,
                  ones_row=ones_row, ones_col_r=ones_col_r, ident=ident,
                  identb=identb, identr=identr, magicv=magicv, swb=swb,
                  cvec=cvec,
                  wqbf=wqbf)

    sts = [St(), St()]
    ks = int(os.environ.get("KSKIP", "5"))
    # interleaved emission: batch b's collectives overlap batch 1-b's compute
    _p1(shared, sts, 0)
    _p2(shared, sts, 0)
    _p1(shared, sts, 1)
    if ks >= 2:
        _p3(shared, sts, 0, decode_b=None)
    if ks >= 3:
        _p4a(shared, sts, 0)
    _p2(shared, sts, 1)
    if ks >= 3:
        _p4b(shared, sts, 0)
    if ks >= 2:
        _p3(shared, sts, 1, decode_b=0 if ks >= 4 else None)
    if ks >= 3:
        _p4a(shared, sts, 1)
    if ks >= 4:
        _p5(shared, sts, 0)
    if ks >= 3:
        _p4b(shared, sts, 1)
    if ks >= 4:
        _p5(shared, sts, 1)
    stack.close()


def _p1(shared, sts, b):
    """Load x, per-token int8 quantize (scalar+vector), DRAM roundtrip,
    XBAR transposes into blk[i] (= qxT), R broadcast blocks."""
    nc, tc = shared["nc"], shared["tc"]
    st = sts[b]
    fp32, bf16 = DT.float32, DT.bfloat16
    X = mybir.AxisListType.X
    dram = shared["dram"]
    big = shared["big"]
    magicv = shared["magicv"]

    # shared-tag requests (b1 reuses b0's buffers with WAR rotation)
    st.t["qxT"] = [big.tile([P, T], bf16, tag=f"blk{i}", name=f"blk{i}_{b}")
                   for i in range(NF)]
    st.t["R4"] = [big.tile([P, TB], bf16, tag=f"R4_{t}", name=f"R4_{t}_{b}")
                  for t in range(NTB)]
    st.t["sinv4"] = [big.tile([P, 4], fp32, tag=f"si{t}", name=f"si{t}_{b}")
                     for t in range(NTB)]

    x_in = shared["x_in"]
    d_qx4 = [dram.tile([TB, FD], bf16, tag=f"d_qx{b}_{t}", name=f"d_qx{b}_{t}")
             for t in range(NTB)]
    d_si = dram.tile([NT, P], fp32, tag=f"d_si{b}", name=f"d_si{b}")

    with (
        tc.tile_pool(name=f"p1s{b}", bufs=3) as p1s,
        tc.tile_pool(name=f"xst{b}", bufs=2) as xstage,
        tc.tile_pool(name=f"qbp{b}", bufs=2) as qbp,
        tc.tile_pool(name=f"tqps{b}", bufs=2, space="PSUM") as tqps,
        tc.tile_pool(name=f"bc{b}", bufs=2, space="PSUM") as bcps,
    ):
        for j in range(NT):
            t4 = j // 4
            xt = xstage.tile([P, FD], fp32, tag="xt", name="xt")
            nc.sync.dma_start(
                out=xt[:], in_=x_in[b * T + j * P: b * T + (j + 1) * P, :])
            am = p1s.tile([P, 1], fp32, tag="am", name="am")
            nc.vector.tensor_reduce(am[:], xt[:], X, ALU.max,
                                    apply_absolute_value=True)
            amc = p1s.tile([P, 1], fp32, tag="amc", name="amc")
            nc.gpsimd.tensor_scalar_max(amc[:], am[:], 1e-5)
            sv = p1s.tile([P, 1], fp32, tag="sv", name="sv")
            nc.vector.reciprocal(sv[:], amc[:])
            svec = p1s.tile([P, 1], fp32, tag="svec", name="svec")
            nc.vector.tensor_scalar_mul(svec[:], sv[:], 127.0)
            nc.gpsimd.tensor_scalar_mul(
                st.t["sinv4"][t4][:, j % 4: j % 4 + 1], amc[:], 1.0 / 127.0)
            qb = qbp.tile([P, FD], bf16, tag="qb", name="qb")
            for h in range(2):
                hs = slice(h * (FD // 2), (h + 1) * (FD // 2))
                tq = tqps.tile([P, FD // 2], fp32, tag="tq", name="tq")
                nc.scalar.activation(tq[:], xt[:, hs], ACTF.Identity,
                                     scale=svec[:], bias=magicv[:])
                nc.vector.tensor_scalar_add(qb[:, hs], tq[:], -MAGIC)
            nc.sync.dma_start(out=d_qx4[t4][(j % 4) * P:(j % 4 + 1) * P, :],
                              in_=qb[:])
            if j % 4 == 3:
                # R block: sinv4 -> PE transpose -> DRAM -> row -> broadcast
                pst = bcps.tile([4, P], fp32, tag="sT", name="pst")
                nc.tensor.transpose(pst[:], st.t["sinv4"][t4][:],
                                    shared["ident"][:])
                sT = p1s.tile([4, P], fp32, tag="sTs", name="sTs")
                nc.vector.tensor_copy(sT[:], pst[:])
                nc.sync.dma_start(out=d_si[t4 * 4:(t4 + 1) * 4, :], in_=sT[:])
                srow = p1s.tile([1, TB], fp32, tag="srow", name="srow")
                nc.sync.dma_start(
                    out=srow[:],
                    in_=d_si[t4 * 4:(t4 + 1) * 4, :].rearrange(
                        "a p -> (a p)").unsqueeze(0))
                psr = bcps.tile([P, TB], fp32, tag="bc", name="psr")
                nc.tensor.matmul(psr[:], lhsT=shared["ones_row"][:],
                                 rhs=srow[:], start=True, stop=True)
                nc.vector.tensor_copy(st.t["R4"][t4][:], psr[:])
                for i in range(NF):
                    nc.sync.dma_start(
                        out=st.t["qxT"][i][:, t4 * TB:(t4 + 1) * TB],
                        in_=d_qx4[t4][:, i * P:(i + 1) * P],
                        transpose=True,
                    )


def _p2(shared, sts, b):
    """Projections: QT/KT (weight-stationary), VT -> PE transpose -> V."""
    nc, tc = shared["nc"], shared["tc"]
    st = sts[b]
    fp32, bf16 = DT.float32, DT.bfloat16
    wqbf = shared["wqbf"]
    big = shared["big"]
    qxT = st.t["qxT"]

    st.t["QT"] = [big.tile([P, T], bf16, tag=f"QT{h}", name=f"QT{h}_{b}")
                  for h in range(2)]
    st.t["KT"] = [big.tile([P, T], bf16, tag=f"KT{h}", name=f"KT{h}_{b}")
                  for h in range(2)]
    st.t["V"] = [big.tile([P, OC], bf16, tag=f"V{j}", name=f"V{j}_{b}")
                 for j in range(NT)]

    with (
        tc.tile_pool(name=f"qkps{b}", bufs=2, space="PSUM") as qkps,
        tc.tile_pool(name=f"vtps{b}", bufs=2, space="PSUM") as vtps,
        tc.tile_pool(name=f"vtr{b}", bufs=2, space="PSUM") as vtrps,
        tc.tile_pool(name=f"vts{b}", bufs=2) as vts,
    ):
        for t4 in range(NTB):
            t4s = slice(t4 * TB, (t4 + 1) * TB)
            for wname, dst in (("q", st.t["QT"]), ("k", st.t["KT"])):
                for o in range(2):
                    ps = qkps.tile([P, TB], fp32, tag="qk", name="qk")
                    for i in range(NF):
                        nc.tensor.matmul(
                            ps[:],
                            lhsT=wqbf[wname][i][:, o * P:(o + 1) * P],
                            rhs=qxT[i][:, t4s],
                            start=(i == 0), stop=(i == NF - 1),
                        )
                    nc.vector.tensor_tensor(dst[o][:, t4s], ps[:],
                                            st.t["R4"][t4][:], ALU.mult)
            for dh in range(2):
                ps = vtps.tile([P, TB], fp32, tag="vt", name="vt")
                for i in range(NF):
                    nc.tensor.matmul(
                        ps[:],
                        lhsT=wqbf["v"][i][:, dh * P:(dh + 1) * P],
                        rhs=qxT[i][:, t4s],
                        start=(i == 0), stop=(i == NF - 1),
                    )
                vt = vts.tile([P, TB], bf16, tag="vts", name="vt_sb")
                nc.vector.tensor_scalar_mul(vt[:], ps[:],
                                            shared["swb"][:, 6:7])
                for jj in range(4):
                    j = t4 * 4 + jj
                    pst = vtrps.tile([P, P], bf16, tag="vtr", name="vtr")
                    nc.tensor.transpose(pst[:], vt[:, jj * P:(jj + 1) * P],
                                        shared["identb"][:])
                    nc.vector.tensor_scalar(
                        st.t["V"][j][:, dh * P:(dh + 1) * P], pst[:],
                        st.t["sinv4"][t4][:, jj:jj + 1], None, ALU.mult,
                    )


def _p3(shared, sts, b, decode_b=None):
    """Attention: ss -> exp -> pv; denominators via ex-pair sums + short
    matmuls. Optionally weave the other batch's P5 decode cast-DMAs in."""
    nc, tc = shared["nc"], shared["tc"]
    st = sts[b]
    fp32, bf16 = DT.float32, DT.bfloat16
    cvec = shared["cvec"]
    big = shared["big"]

    st.t["PVT"] = [big.tile([P, T], bf16, tag=f"PVT{h}", name=f"PVT{h}_{b}")
                   for h in range(2)]
    st.t["den"] = [big.tile([1, T], fp32, tag=f"dnr{h}", name=f"dnr{h}_{b}")
                   for h in range(2)]

    with (
        tc.tile_pool(name=f"sps{b}", bufs=(2 if os.environ.get("P3MODE", "0") == "1" else 4), space="PSUM") as sps,
        tc.tile_pool(name=f"pvps{b}", bufs=2, space="PSUM") as pvps,
        tc.tile_pool(name=f"dnps{b}", bufs=1, space="PSUM") as dnps,
        tc.tile_pool(name=f"expp{b}", bufs=4) as expp,
        tc.tile_pool(name=f"exP{b}", bufs=9) as exP,
    ):
        if decode_b is not None:
            _p5pre(shared, sts, decode_b)
            dq = sts[decode_b].t["qaTg_bf"]
            d_g = sts[decode_b].t["d_qaTg"]

        QT, KT, V = st.t["QT"], st.t["KT"], st.t["V"]
        p3mode = int(os.environ.get("P3MODE", "0"))
        groups = [(hl, t1b) for hl in range(2) for t1b in range(NTB)]
        pend = []
        for g, (hl, t1b) in enumerate(groups):
            t1s = slice(t1b * TB, (t1b + 1) * TB)
            pv = pvps.tile([P, TB], fp32, tag="pv", name="pv")
            eng = nc.vector if g % 2 == 0 else nc.gpsimd
            pairs = []
            # software-pipelined: ss/exp of pair p+1 are emitted before the
            # pv matmuls of pair p, so the PE never heads-of-line-blocks on
            # the scalar engine's exp.
            exs = [None, None]  # ex tiles of pairs p-1, p (rotating)
            for p in range(NT // 2):
                j0, j1 = 2 * p, 2 * p + 1
                if p3mode == 1:
                    ss = sps.tile([P, 2 * TB], fp32, tag="ss", name="ss")
                    nc.tensor.matmul(
                        ss[:, 0:TB], lhsT=KT[hl][:, j0 * P:(j0 + 1) * P],
                        rhs=QT[hl][:, t1s], start=True, stop=True,
                    )
                    nc.tensor.matmul(
                        ss[:, TB:2 * TB], lhsT=KT[hl][:, j1 * P:(j1 + 1) * P],
                        rhs=QT[hl][:, t1s], start=True, stop=True,
                    )
                    ex = expp.tile([P, 2 * TB], bf16, tag="ex", name="ex")
                    nc.scalar.activation(ex[:], ss[:], ACTF.Exp,
                                         scale=cvec[:])
                else:
                    ex = expp.tile([P, 2 * TB], bf16, tag="ex", name="ex")
                    for d, j in enumerate((j0, j1)):
                        ss = sps.tile([P, TB], fp32, tag="ss", name="ss")
                        nc.tensor.matmul(
                            ss[:], lhsT=KT[hl][:, j * P:(j + 1) * P],
                            rhs=QT[hl][:, t1s], start=True, stop=True,
                        )
                        nc.scalar.activation(ex[:, d * TB:(d + 1) * TB],
                                             ss[:], ACTF.Exp, scale=cvec[:])
                exs[p % 2] = ex
                if p >= 1:
                    _emit_pv(nc, st, pvps, pv, V, hl, exs[(p - 1) % 2],
                             2 * (p - 1), eng, exP, pairs)
            _emit_pv(nc, st, pvps, pv, V, hl, exs[(NT // 2 - 1) % 2],
                     NT - 2, eng, exP, pairs)
            # delay dn by one group so PE never waits on the pair adds
            pend.append((pairs, pv, hl, t1b, eng))
            if len(pend) == 2:
                _emit_dn(shared, st, dnps, pend.pop(0))
            if decode_b is not None and g % 2 == 1:
                i0 = (g // 2) * 4
                for i in range(i0, i0 + 4):
                    nc.gpsimd.dma_start(out=dq[i][:],
                                        in_=d_g[i * P:(i + 1) * P, :])
        while pend:
            _emit_dn(shared, st, dnps, pend.pop(0))


def _emit_pv(nc, st, pvps, pv, V, hl, ex, j0, eng, exP, pairs):
    """pv matmuls for k-tiles j0, j0+1 (the two halves of ex) + pair-sum."""
    for d, j in enumerate((j0, j0 + 1)):
        nc.tensor.matmul(
            pv[:], lhsT=V[j][:, hl * P:(hl + 1) * P],
            rhs=ex[:, d * TB:(d + 1) * TB],
            start=(j == 0), stop=(j == NT - 1),
        )
    pr = exP.tile([P, TB], DT.float32r, tag="exP", name="exP")
    eng.tensor_tensor(pr[:], ex[:, 0:TB], ex[:, TB:2 * TB], ALU.add)
    pairs.append(pr)


def _emit_dn(shared, st, dnps, item):
    nc = shared["nc"]
    pairs, pv, hl, t1b, eng = item
    t1s = slice(t1b * TB, (t1b + 1) * TB)
    dn = dnps.tile([1, TB], DT.float32, tag="dn", name="dn")
    if os.environ.get("DNTREE", "1") == "1":
        # reduce 8 pairs -> 4 -> 2 -> 1 on the group's vector engine, then a
        # single ones-matmul row-sum.
        lvl = list(pairs)
        while len(lvl) > 1:
            nxt = []
            for a, b2 in zip(lvl[0::2], lvl[1::2]):
                eng.tensor_tensor(a[:], a[:], b2[:], ALU.add)
                nxt.append(a)
            lvl = nxt
        nc.tensor.matmul(dn[:], lhsT=shared["ones_col_r"][:], rhs=lvl[0][:],
                         start=True, stop=True)
    else:
        for pi, pr in enumerate(pairs):
            nc.tensor.matmul(
                dn[:], lhsT=shared["ones_col_r"][:], rhs=pr[:],
                start=(pi == 0), stop=(pi == len(pairs) - 1),
            )
    nc.vector.tensor_copy(st.t["PVT"][hl][:, t1s], pv[:])
    nc.vector.tensor_copy(st.t["den"][hl][:, t1s], dn[:])


def _p4a(shared, sts, b):
    """den -> denT -> 1/denT; PVT transpose+normalize -> PVn; rmax; AR."""
    nc, tc = shared["nc"], shared["tc"]
    st = sts[b]
    fp32, bf16 = DT.float32, DT.bfloat16
    X = mybir.AxisListType.X
    dram = shared["dram"]
    big = shared["big"]

    st.t["PVn"] = [big.tile([P, OC], bf16, tag=f"PVn{j}", name=f"PVn{j}_{b}")
                   for j in range(NT)]
    st.t["rmax"] = big.tile([P, NT], fp32, tag="rmax", name=f"rmax_{b}")
    st.t["sqa"] = big.tile([P, NT], fp32, tag="sqa", name=f"sqa_{b}")
    rdenT = [big.tile([P, NT], fp32, tag=f"rdT{h}", name=f"rdT{h}_{b}")
             for h in range(2)]

    with (
        tc.tile_pool(name=f"tps{b}", bufs=2, space="PSUM") as tps,
        tc.tile_pool(name=f"p4s{b}", bufs=2) as p4s,
    ):
        for hl in range(2):
            d_den = dram.tile([1, T], fp32, tag=f"d_den{b}{hl}",
                              name=f"d_den{b}{hl}")
            nc.sync.dma_start(out=d_den[:], in_=st.t["den"][hl][:])
            den16 = p4s.tile([NT, P], fp32, tag="den16", name="den16")
            nc.sync.dma_start(
                out=den16[:], in_=d_den.rearrange("o (j p) -> (o j) p", p=P))
            pstd = tps.tile([P, NT], fp32, tag="dT", name="pstd")
            nc.tensor.transpose(pstd[:], den16[:],
                                shared["ident"][0:NT, 0:NT])
            dt_ = p4s.tile([P, NT], fp32, tag="dT2", name="dt_")
            nc.vector.tensor_copy(dt_[:], pstd[:])
            nc.vector.reciprocal(rdenT[hl][:], dt_[:])
        for j in range(NT):
            for hl in range(2):
                pst = tps.tile([P, P], bf16, tag="t", name="pst4")
                nc.tensor.transpose(pst[:],
                                    st.t["PVT"][hl][:, j * P:(j + 1) * P],
                                    shared["identb"][:])
                nc.vector.tensor_scalar(
                    st.t["PVn"][j][:, hl * P:(hl + 1) * P], pst[:],
                    rdenT[hl][:, j:j + 1], None, ALU.mult,
                )
            nc.vector.tensor_reduce(st.t["rmax"][:, j:j + 1],
                                    st.t["PVn"][j][:], X, ALU.max,
                                    apply_absolute_value=True)
        d_rm_in = dram.tile([P, NT], fp32, tag=f"d_rmi{b}", name=f"d_rmi{b}")
        d_rm_out = dram.tile([P, NT], fp32, tag=f"d_rmo{b}", name=f"d_rmo{b}")
        nc.sync.dma_start(out=d_rm_in[:], in_=st.t["rmax"][:])
        nc.gpsimd.collective_compute(
            "AllReduce", ALU.max, replica_groups=RG,
            ins=[d_rm_in.opt()], outs=[d_rm_out.opt()],
        )
        st.t["d_rm_out"] = d_rm_out


def _p4b(shared, sts, b):
    """AR read -> scales; quantize PVn -> qa bf16 -> PE transpose -> int8
    qaT -> DRAM -> AllGather."""
    nc, tc = shared["nc"], shared["tc"]
    st = sts[b]
    fp32, bf16, i8 = DT.float32, DT.bfloat16, DT.int8
    dram = shared["dram"]
    big = shared["big"]

    st.t["Ry"] = big.tile([P, NT], fp32, tag=f"Ry{b}", name=f"Ry{b}")
    qaT8 = [big.tile([P, T], i8, tag=f"qaT8{d}", name=f"qaT8{d}_{b}")
            for d in range(2)]

    with (
        tc.tile_pool(name=f"q4s{b}", bufs=2) as q4s,
        tc.tile_pool(name=f"q4ps{b}", bufs=2, space="PSUM") as q4ps,
    ):
        rmg = q4s.tile([P, NT], fp32, tag="rmg", name="rmg", bufs=1)
        nc.sync.dma_start(out=rmg[:], in_=st.t["d_rm_out"][:])
        mxt = q4s.tile([P, NT], fp32, tag="mxt", name="mxt", bufs=1)
        nc.vector.tensor_scalar_max(mxt[:], rmg[:], 1e-5)
        rc = q4s.tile([P, NT], fp32, tag="rc", name="rc", bufs=1)
        nc.vector.reciprocal(rc[:], mxt[:])
        nc.vector.tensor_scalar_mul(st.t["sqa"][:], rc[:], 127.0)
        nc.vector.tensor_scalar(st.t["Ry"][:], mxt[:], shared["swb"][:, 7:8],
                                1.0 / 127.0, ALU.mult, ALU.mult)
        for j in range(NT):
            tq = q4s.tile([P, OC], fp32, tag="tq4", name="tq4")
            nc.scalar.activation(tq[:], st.t["PVn"][j][:], ACTF.Identity,
                                 scale=st.t["sqa"][:, j:j + 1],
                                 bias=shared["magicv"][:])
            qa = q4s.tile([P, OC], bf16, tag="qa", name="qa")
            nc.vector.tensor_scalar_add(qa[:], tq[:], -MAGIC)
            for dl in range(2):
                pst = q4ps.tile([P, P], bf16, tag="qt", name="qt")
                nc.tensor.transpose(pst[:], qa[:, dl * P:(dl + 1) * P],
                                    shared["identb"][:])
                nc.vector.tensor_copy(qaT8[dl][:, j * P:(j + 1) * P], pst[:])
        d_qaT = dram.tile([OC, T], i8, tag=f"d_qaT{b}", name=f"d_qaT{b}")
        for dl in range(2):
            nc.sync.dma_start(out=d_qaT[dl * P:(dl + 1) * P, :],
                              in_=qaT8[dl][:])
        d_qaTg = dram.tile([FD, T], i8, tag=f"d_qaTg{b}", name=f"d_qaTg{b}",
                           addr_space="Shared")
        nc.gpsimd.collective_compute(
            "AllGather", ALU.bypass, replica_groups=RG,
            ins=[d_qaT.opt()], outs=[d_qaTg.opt()],
        )
        st.t["d_qaTg"] = d_qaTg


def _p5pre(shared, sts, b):
    """Request decode destinations: reuse the qx blk tile tags."""
    st = sts[b]
    big = shared["big"]
    st.t["qaTg_bf"] = [
        big.tile([P, T], DT.bfloat16, tag=f"blk{i}", name=f"qg{b}{i}")
        for i in range(NF)
    ]


def _p5(shared, sts, b):
    """out_proj on decoded bf16 activations; transpose yT -> y; DMA out."""
    nc, tc = shared["nc"], shared["tc"]
    st = sts[b]
    fp32 = DT.float32
    wqbf = shared["wqbf"]

    decode_inline = "qaTg_bf" not in st.t
    if decode_inline:
        _p5pre(shared, sts, b)
    dq = st.t["qaTg_bf"]

    yts_cm = tc.tile_pool(name=f"yts{b}", bufs=4)
    yts = yts_cm.__enter__()
    ynp_cm = tc.tile_pool(name=f"ynp{b}", bufs=3)
    ynp = ynp_cm.__enter__()
    # i-outer with 8 PSUM accumulators: each decoded tile i feeds all 8
    # matmuls immediately, so decode cast-DMAs overlap the accumulation.
    p5mode = int(os.environ.get("P5MODE", "0"))
    yT = [[None] * NTB for _ in range(2)]
    if p5mode == 1:
        ops_cm = tc.tile_pool(name=f"ops{b}", bufs=1, space="PSUM")
        ops = ops_cm.__enter__()
        acc = [[ops.tile([P, TB], fp32, tag=f"a{o}{t4}", name=f"a{o}{t4}")
                for t4 in range(NTB)] for o in range(2)]
        for i in range(NF):
            if decode_inline:
                nc.gpsimd.dma_start(out=dq[i][:],
                                    in_=st.t["d_qaTg"][i * P:(i + 1) * P, :])
            for o in range(2):
                for t4 in range(NTB):
                    nc.tensor.matmul(
                        acc[o][t4][:],
                        lhsT=wqbf["o"][i][:, o * P:(o + 1) * P],
                        rhs=dq[i][:, t4 * TB:(t4 + 1) * TB],
                        start=(i == 0), stop=(i == NF - 1),
                    )
        for o in range(2):
            for t4 in range(NTB):
                yT[o][t4] = yts.tile([P, TB], DT.float32r, tag=f"yT{o}",
                                     name=f"yT{o}{t4}")
                nc.vector.tensor_copy(yT[o][t4][:], acc[o][t4][:])
        ops_cm.__exit__(None, None, None)
    else:
        if decode_inline:
            for i in range(NF):
                nc.gpsimd.dma_start(
                    out=dq[i][:],
                    in_=st.t["d_qaTg"][i * P:(i + 1) * P, :])
        ops_cm = tc.tile_pool(name=f"ops{b}", bufs=2, space="PSUM")
        ops = ops_cm.__enter__()
        for t4 in range(NTB):
            pso = [ops.tile([P, TB], fp32, tag=f"o{o}", name=f"po{o}")
                   for o in range(2)]
            for i in range(NF):
                for o in range(2):
                    nc.tensor.matmul(
                        pso[o][:],
                        lhsT=wqbf["o"][i][:, o * P:(o + 1) * P],
                        rhs=dq[i][:, t4 * TB:(t4 + 1) * TB],
                        start=(i == 0), stop=(i == NF - 1),
                    )
            for o in range(2):
                yT[o][t4] = yts.tile([P, TB], DT.float32r, tag=f"yT{o}",
                                     name=f"yT{o}{t4}")
                nc.vector.tensor_copy(yT[o][t4][:], pso[o][:])
        ops_cm.__exit__(None, None, None)

    tps2_cm = tc.tile_pool(name=f"tps2{b}", bufs=2, space="PSUM")
    tps2 = tps2_cm.__enter__()
    for j in range(NT):
        t4, jj = j // 4, j % 4
        yn = ynp.tile([P, OC], fp32, tag="yn", name="yn")
        for o in range(2):
            pst = tps2.tile([P, P], DT.float32r, tag="t2", name="pst5")
            nc.tensor.transpose(
                pst[:], yT[o][t4][:, jj * P:(jj + 1) * P],
                shared["identr"][:])
            nc.vector.tensor_scalar(
                yn[:, o * P:(o + 1) * P], pst[:].bitcast(fp32),
                st.t["Ry"][:, j:j + 1], None, ALU.mult,
            )
        nc.sync.dma_start(
            out=shared["y_out"][b, j * P:(j + 1) * P, :], in_=yn[:])
    tps2_cm.__exit__(None, None, None)
    ynp_cm.__exit__(None, None, None)
    yts_cm.__exit__(None, None, None)


# ---------------------------------------------------------------------------
_CACHE = {}


def _get_nc():
    if "nc" not in _CACHE:
        _CACHE["nc"] = build_kernel()
    return _CACHE["nc"]


def prepare_in_maps(x, w_q, w_k, w_v, w_o):
    xf = np.ascontiguousarray(np.asarray(x, np.float32).reshape(BT, FD))
    ws = {"q": w_q, "k": w_k, "v": w_v, "o": w_o}
    in_maps = []
    for c in range(N_CORES):
        m = {"x": xf}
        for k, w in ws.items():
            sl = np.asarray(w, np.float32)[c * OC:(c + 1) * OC, :]
            m[f"w{k}T"] = np.ascontiguousarray(sl.T)
        in_maps.append(m)
    return in_maps


def kernel(x, w_q, w_k, w_v, w_o):
    nc = _get_nc()
    in_maps = prepare_in_maps(x, w_q, w_k, w_v, w_o)
    last_err = None
    for _attempt in range(4):
        try:
            res = run_bass_kernel_spmd(nc, in_maps, list(range(N_CORES)))
            break
        except Exception as e:  # sporadic device-unrecoverable; retry
            last_err = e
            import time as _time
            _time.sleep(2.0)
    else:
        raise last_err
    outs = [res.results[c]["y"] for c in range(N_CORES)]  # [B, T, OC] each
    y = np.concatenate(outs, axis=2)  # [B, T, FD]
    return np.ascontiguousarray(y.astype(np.float32))
